# revision 41
# baseline (speedup 1.0000x reference)
"""Distributed Trainium2 kernel for nn_Attention (B=2, N=2048, C=1024, H=16, HD=64).

Sharding: sequence-parallel. Core c owns batch b=c//4 and query rows
[512*(c%4), 512*(c%4+1)).  Each core computes q/k/v for its own rows,
RoPEs q and k, AllGathers k^T and v (within its 4-core batch group),
then computes attention + projection for its row slice.  Outputs are
disjoint row slices of the final [B, N, C] tensor — no reduction needed.

All matmuls run in float32r (full-rate fp32).  Weights are pre-transposed
on the host so every matmul operand has its natural layout on device.
Attention is computed transposed (S^T = k^T q) so softmax denominators
come from an appended ones-column in v, and no on-device transposes are
ever needed.
"""

import sys

if "/opt/trn_rl_repo" not in sys.path:
    sys.path.insert(0, "/opt/trn_rl_repo")

import numpy as np

B, N, C = 2, 2048, 1024
H, HD = 16, 64
NCORES = 8
GB = 4          # cores per batch (replica group size)
NS = N // GB    # 512 rows per core
SC = HD ** -0.5  # attention scale


def build(mock_ag=False):
    import concourse.bass as bass
    import concourse.mybir as mybir
    import concourse.tile as tile
    from concourse import bacc

    f32 = mybir.dt.float32
    f32r = mybir.dt.float32r
    AF = mybir.ActivationFunctionType

    nc = bacc.Bacc(None, target_bir_lowering=False, num_devices=NCORES)

    # ---- per-core external inputs (host pre-shards / pre-transposes) ----
    xT = nc.declare_dram_parameter("xT", [C, NS], f32r, isOutput=False)
    wqkT = nc.declare_dram_parameter("wqkT", [C, 2 * C], f32r, isOutput=False)
    wvT = nc.declare_dram_parameter("wvT", [C, C], f32r, isOutput=False)
    wpT = nc.declare_dram_parameter("wpT", [C, C], f32r, isOutput=False)
    cos2 = nc.declare_dram_parameter("cos2", [128, NS], f32, isOutput=False)
    sins2 = nc.declare_dram_parameter("sins2", [128, NS], f32, isOutput=False)
    biasb = nc.declare_dram_parameter("biasb", [128, C], f32, isOutput=False)
    out = nc.declare_dram_parameter("out", [NS, C], f32, isOutput=True)

    groups = [list(range(GB)), list(range(GB, 2 * GB))]

    def mm(out_ap, lhsT_ap, rhs_ap, start, stop):
        nc.tensor.matmul(out_ap, lhsT_ap, rhs_ap, start=start, stop=stop)

    from contextlib import ExitStack

    with tile.TileContext(nc) as tc:
        with ExitStack() as stack:
            ep = stack.enter_context
            ep(nc.allow_low_precision(reason="f32r rounding of fp32 matmul inputs"))
            dramp = ep(tc.tile_pool(name="dram", bufs=1, space="DRAM"))
            constp = ep(tc.tile_pool(name="const", bufs=1))
            xtp = ep(tc.tile_pool(name="xTp", bufs=1))
            qtp = ep(tc.tile_pool(name="qTp", bufs=1))
            atp = ep(tc.tile_pool(name="aTp", bufs=1))
            wtsp = ep(tc.tile_pool(name="wts", bufs=20))
            ktmpp = ep(tc.tile_pool(name="ktmp", bufs=3))
            ropep = ep(tc.tile_pool(name="ropet", bufs=3))
            kheadp = ep(tc.tile_pool(name="khead", bufs=2))
            ptp = ep(tc.tile_pool(name="pTp", bufs=3))
            vhp_p = ep(tc.tile_pool(name="vhp", bufs=4))
            smallp = ep(tc.tile_pool(name="small", bufs=4))
            outp = ep(tc.tile_pool(name="outsb", bufs=3))
            ps_mm = ep(tc.tile_pool(name="ps_mm", bufs=2, space="PSUM"))
            ps_s = ep(tc.tile_pool(name="ps_s", bufs=2, space="PSUM"))
            ps_av = ep(tc.tile_pool(name="ps_av", bufs=2, space="PSUM"))

            # ---- internal DRAM for collectives (split by head half) ----
            k_inh, k_gathh, v_inh, v_gathh = [], [], [], []
            for s in range(2):
                k_inh.append(dramp.tile([C // 2, NS], f32r, name=f"k_in{s}"))
                k_gathh.append(
                    dramp.tile([GB, C // 2, NS], f32r, name=f"k_gath{s}")
                )
                v_inh.append(
                    dramp.tile([NS, 8, HD + 1], f32r, name=f"v_in{s}")
                )
                v_gathh.append(
                    dramp.tile([GB, NS, 8, HD + 1], f32r, name=f"v_gath{s}")
                )

            # ---- constants / persistent loads ----
            cos_sb = constp.tile([128, NS], f32, name="cos_sb")
            nc.sync.dma_start(cos_sb[:, :], cos2[:, :])
            sin_sb = constp.tile([128, NS], f32, name="sin_sb")
            nc.sync.dma_start(sin_sb[:, :], sins2[:, :])
            bias_sb = constp.tile([128, C], f32, name="bias_sb")
            nc.sync.dma_start(bias_sb[:, :], biasb[:, :])
            onesf = constp.tile([128, 64], f32, name="onesf")
            nc.vector.memset(onesf[:, :], 1.0)

            xT_sb = xtp.tile([128, 8, NS], f32r, name="xT_sb")
            for cc in range(8):
                nc.sync.dma_start(
                    xT_sb[:, cc, :], xT[cc * 128:(cc + 1) * 128, :]
                )

            qT_sb = qtp.tile([128, 8, NS], f32r, name="qT_sb")
            aT_sb = atp.tile([128, 8, NS], f32r, name="aT_sb")

            def rope_chunk(psum, dst):
                """dst = psum*cos + rot32(psum)*signed_sin, all [128, NS]."""
                tmp = ropep.tile([128, NS], f32, name="tmp", tag="ropetmp")
                for lo in (0, 64):
                    nc.vector.tensor_mul(
                        tmp[lo:lo + 32, :],
                        psum[lo + 32:lo + 64, :],
                        sin_sb[lo:lo + 32, :],
                    )
                    nc.vector.tensor_mul(
                        tmp[lo + 32:lo + 64, :],
                        psum[lo:lo + 32, :],
                        sin_sb[lo + 32:lo + 64, :],
                    )
                nc.vector.tensor_mul(dst, psum, cos_sb[:, :])
                nc.vector.tensor_add(dst, dst, tmp[:, :])

            # ---- v (natural [i, dv]) and k^T, in head halves; AG each ----
            def ag(in_t, out_t, tag):
                if mock_ag:
                    for r in range(GB):
                        nc.gpsimd.dma_start(out_t[r, 0:32], in_t[0:32])
                else:
                    nc.gpsimd.collective_compute(
                        "AllGather",
                        mybir.AluOpType.bypass,
                        replica_groups=groups,
                        ins=[in_t.opt()],
                        outs=[out_t.opt()],
                    )

            for s in range(2):  # head half s: heads 8s..8s+7
                wv_tiles = []
                for cc in range(8):
                    w = wtsp.tile([128, 512], f32r, name="w", tag="wts")
                    nc.sync.dma_start(
                        w[:, :],
                        wvT[cc * 128:(cc + 1) * 128, s * 512:(s + 1) * 512],
                    )
                    wv_tiles.append(w)
                wk_tiles = []
                for cc in range(8):
                    w = wtsp.tile([128, 4, 128], f32r, name="w", tag="wts")
                    nc.scalar.dma_start(
                        w[:, :, :],
                        wqkT[
                            cc * 128:(cc + 1) * 128,
                            C + s * 512:C + (s + 1) * 512,
                        ].rearrange("p (m f) -> p m f", f=128),
                    )
                    wk_tiles.append(w)
                # v half
                for ic in range(4):
                    rows = slice(ic * 128, (ic + 1) * 128)
                    psum = ps_mm.tile([128, NS], f32, name="psum", tag="mm")
                    for cc in range(8):
                        mm(psum[:, :], xT_sb[:, cc, rows], wv_tiles[cc][:, :],
                           cc == 0, cc == 7)
                    vsb = outp.tile([128, 8, HD + 1], f32r, name="vsb", tag="osb")
                    nc.vector.tensor_copy(vsb[:, :, HD], onesf[:, 0:8])
                    nc.vector.tensor_copy(
                        vsb[:, :, 0:HD],
                        psum[:, :].rearrange("p (h d) -> p h d", d=HD),
                    )
                    nc.scalar.dma_start(v_inh[s][rows, :, :], vsb[:, :, :])
                ag(v_inh[s], v_gathh[s], f"v{s}")
                # k half
                for ml in range(4):
                    psum = ps_mm.tile([128, NS], f32, name="psum", tag="mm")
                    for cc in range(8):
                        mm(psum[:, :], wk_tiles[cc][:, ml, :], xT_sb[:, cc, :],
                           cc == 0, cc == 7)
                    kc = ktmpp.tile([128, NS], f32r, name="kc", tag="kc")
                    rope_chunk(psum[:, :], kc[:, :])
                    nc.scalar.dma_start(
                        k_inh[s][ml * 128:(ml + 1) * 128, :], kc[:, :]
                    )
                ag(k_inh[s], k_gathh[s], f"k{s}")

            # ---- q^T group (dq chunks 0..7) + rope, overlaps the gathers ----
            wq_tiles = {}
            for qh in range(2):
                for cc in range(8):
                    w = wtsp.tile([128, 4, 128], f32r, name="w", tag="wts")
                    nc.scalar.dma_start(
                        w[:, :, :],
                        wqkT[
                            cc * 128:(cc + 1) * 128, qh * 512:(qh + 1) * 512
                        ].rearrange("p (m f) -> p m f", f=128),
                    )
                    wq_tiles[(qh, cc)] = w
            for m in range(8):
                psum = ps_mm.tile([128, NS], f32, name="psum", tag="mm")
                for cc in range(8):
                    mm(psum[:, :], wq_tiles[(m // 4, cc)][:, m % 4, :],
                       xT_sb[:, cc, :], cc == 0, cc == 7)
                rope_chunk(psum[:, :], qT_sb[:, m, :])

            # ---- attention, head pairs (flash-style over key chunks) ----
            vg = {}
            for hp in range(H // 2):  # heads 2*hp, 2*hp+1
                if hp % 4 == 0:  # prefetch v for heads [8*g, 8*(g+1))
                    g = hp // 4
                    for r in range(GB):
                        vt = vhp_p.tile(
                            [128, GB, 8, HD + 1], f32r, name="vt", tag="vt"
                        )
                        for half in range(2):
                            eng = [nc.gpsimd, nc.sync][(r + half) % 2]
                            eng.dma_start(
                                vt[:, half * 2:(half + 1) * 2, :, :],
                                v_gathh[g][
                                    r, half * 256:(half + 1) * 256, :, :
                                ].rearrange("(a p) h d -> p a h d", p=128),
                            )
                        vg[r] = vt
                kh = kheadp.tile([128, GB, NS], f32r, name="kh", tag="khead")
                kh_engines = [nc.gpsimd, nc.sync, nc.gpsimd, nc.sync]
                for r in range(GB):
                    kh_engines[r].dma_start(
                        kh[:, r, :],
                        k_gathh[hp // 4][
                            r, (hp % 4) * 128:(hp % 4 + 1) * 128, :
                        ],
                    )
                for sub in range(2):  # head h = 2*hp + sub at partitions sub*64
                    h = 2 * hp + sub
                    lo = sub * 64
                    q_ap = qT_sb[lo:lo + 64, hp, :]
                    po = ps_av.tile([HD + 1, NS], f32, name="po", tag="av")
                    for jp in range(8):  # pairs of key chunks
                        jc0 = 2 * jp
                        ps2 = ps_s.tile([128, 2, NS], f32, name="ps2", tag="sc")
                        for u in range(2):
                            jc = jc0 + u
                            r, jl = jc // 4, jc % 4
                            mm(ps2[:, u, :],
                               kh[lo:lo + 64, r, jl * 128:(jl + 1) * 128],
                               q_ap, True, True)
                        pt = ptp.tile([128, 2, NS], f32r, name="pt", tag="pT")
                        nc.scalar.activation(
                            pt[:, :, :], ps2[:, :, :], AF.Exp, scale=SC
                        )
                        for u in range(2):
                            jc = jc0 + u
                            r, jl = jc // 4, jc % 4
                            mm(po[:, :],
                               vg[r][:, jl, 2 * (hp % 4) + sub, :],
                               pt[:, u, :], jc == 0, jc == 15)
                    # normalize: reciprocal of denom row, gpsimd broadcast
                    recip = smallp.tile([1, NS], f32, name="recip", tag="recip")
                    nc.vector.reciprocal(recip[:, :], po[HD:HD + 1, :])
                    rb = smallp.tile([64, NS], f32, name="rb", tag="rb")
                    nc.gpsimd.partition_broadcast(rb[:, :], recip[:, :])
                    nc.vector.tensor_mul(
                        aT_sb[lo:lo + 64, hp, :], po[0:HD, :], rb[:, :]
                    )

            # ---- projection ----
            wp_tiles = {}
            for nn in range(2):
                for cc in range(8):
                    w = wtsp.tile([128, 512], f32r, name="w", tag="wts")
                    nc.sync.dma_start(
                        w[:, :],
                        wpT[cc * 128:(cc + 1) * 128, nn * 512:(nn + 1) * 512],
                    )
                    wp_tiles[(nn, cc)] = w
            for ic in range(4):
                rows = slice(ic * 128, (ic + 1) * 128)
                for nn in range(2):
                    psum = ps_mm.tile([128, NS], f32, name="psum", tag="mm")
                    for cc in range(8):
                        mm(psum[:, :], aT_sb[:, cc, rows],
                           wp_tiles[(nn, cc)][:, :], cc == 0, cc == 7)
                    osb = outp.tile([128, 512], f32, name="osb", tag="osb")
                    nc.vector.tensor_add(
                        osb[:, :], psum[:, :], bias_sb[:, nn * 512:(nn + 1) * 512]
                    )
                    nc.sync.dma_start(out[rows, nn * 512:(nn + 1) * 512], osb[:, :])

    nc.compile()
    return nc


_NC_CACHE = {}


def _get_nc():
    if "nc" not in _NC_CACHE:
        _NC_CACHE["nc"] = build()
    return _NC_CACHE["nc"]


def make_in_maps(x, cos, sin, qkv_w, proj_w, proj_b):
    x = np.asarray(x, np.float32)
    cos = np.asarray(cos, np.float32)
    sin = np.asarray(sin, np.float32)
    qkv_w = np.asarray(qkv_w, np.float32)
    proj_w = np.asarray(proj_w, np.float32)
    proj_b = np.asarray(proj_b, np.float32)

    wqkT = np.ascontiguousarray(qkv_w[: 2 * C].T)        # [C, 2C]
    wvT = np.ascontiguousarray(qkv_w[2 * C:].T)          # [C, C]
    wpT = np.ascontiguousarray(proj_w.T)                 # [C, C]
    biasb = np.ascontiguousarray(np.broadcast_to(proj_b, (128, C)))
    sign = np.concatenate([-np.ones(32, np.float32), np.ones(32, np.float32)])

    in_maps = []
    for c in range(NCORES):
        b, r = c // GB, c % GB
        rows = slice(r * NS, (r + 1) * NS)
        xTc = np.ascontiguousarray(x[b, rows].T)          # [C, NS]
        cosT = cos[rows].T                                # [HD, NS]
        sinsT = (sin[rows] * sign).T                      # [HD, NS] signed
        cos2v = np.ascontiguousarray(np.concatenate([cosT, cosT], 0))   # [128, NS]
        sins2v = np.ascontiguousarray(np.concatenate([sinsT, sinsT], 0))
        in_maps.append(
            {
                "xT": xTc,
                "wqkT": wqkT,
                "wvT": wvT,
                "wpT": wpT,
                "cos2": cos2v,
                "sins2": sins2v,
                "biasb": biasb,
            }
        )
    return in_maps


def assemble(results):
    out = np.empty((B, N, C), np.float32)
    for c in range(NCORES):
        b, r = c // GB, c % GB
        out[b, r * NS:(r + 1) * NS] = results[c]["out"]
    return out


def kernel(x, cos, sin, qkv_w, proj_w, proj_b):
    from concourse.bass_utils import run_bass_kernel_spmd

    nc = _get_nc()
    in_maps = make_in_maps(x, cos, sin, qkv_w, proj_w, proj_b)
    res = run_bass_kernel_spmd(nc, in_maps, core_ids=list(range(NCORES)))
    return assemble(res.results)


# revision 51
# speedup vs baseline: 1.0066x; 1.0066x over previous
"""Distributed Trainium2 kernel for nn_Attention (B=2, N=2048, C=1024, H=16, HD=64).

Sharding: sequence-parallel. Core c owns batch b=c//4 and query rows
[512*(c%4), 512*(c%4+1)).  Each core computes q/k/v for its own rows,
RoPEs q and k, AllGathers k^T and v (within its 4-core batch group),
then computes attention + projection for its row slice.  Outputs are
disjoint row slices of the final [B, N, C] tensor — no reduction needed.

All matmuls run in float32r (full-rate fp32).  Weights are pre-transposed
on the host so every matmul operand has its natural layout on device.
Attention is computed transposed (S^T = k^T q) so softmax denominators
come from an appended ones-column in v, and no on-device transposes are
ever needed.
"""

import sys

if "/opt/trn_rl_repo" not in sys.path:
    sys.path.insert(0, "/opt/trn_rl_repo")

import numpy as np

B, N, C = 2, 2048, 1024
H, HD = 16, 64
NCORES = 8
GB = 4          # cores per batch (replica group size)
NS = N // GB    # 512 rows per core
SC = HD ** -0.5  # attention scale


def build(mock_ag=False):
    import concourse.bass as bass
    import concourse.mybir as mybir
    import concourse.tile as tile
    from concourse import bacc

    f32 = mybir.dt.float32
    f32r = mybir.dt.float32r
    AF = mybir.ActivationFunctionType

    nc = bacc.Bacc(None, target_bir_lowering=False, num_devices=NCORES)

    # ---- per-core external inputs (host pre-shards / pre-transposes) ----
    xT = nc.declare_dram_parameter("xT", [C, NS], f32r, isOutput=False)
    wqkT = nc.declare_dram_parameter("wqkT", [C, 2 * C], f32r, isOutput=False)
    wvT = nc.declare_dram_parameter("wvT", [C, C], f32r, isOutput=False)
    wpT = nc.declare_dram_parameter("wpT", [C, C], f32r, isOutput=False)
    cos2 = nc.declare_dram_parameter("cos2", [128, NS], f32, isOutput=False)
    sins2 = nc.declare_dram_parameter("sins2", [128, NS], f32, isOutput=False)
    biasb = nc.declare_dram_parameter("biasb", [128, C], f32, isOutput=False)
    out = nc.declare_dram_parameter("out", [NS, C], f32, isOutput=True)

    groups = [list(range(GB)), list(range(GB, 2 * GB))]

    def mm(out_ap, lhsT_ap, rhs_ap, start, stop):
        nc.tensor.matmul(out_ap, lhsT_ap, rhs_ap, start=start, stop=stop)

    from contextlib import ExitStack

    with tile.TileContext(nc) as tc:
        with ExitStack() as stack:
            ep = stack.enter_context
            ep(nc.allow_low_precision(reason="f32r rounding of fp32 matmul inputs"))
            dramp = ep(tc.tile_pool(name="dram", bufs=1, space="DRAM"))
            constp = ep(tc.tile_pool(name="const", bufs=1))
            xtp = ep(tc.tile_pool(name="xTp", bufs=1))
            qtp = ep(tc.tile_pool(name="qTp", bufs=1))
            atp = ep(tc.tile_pool(name="aTp", bufs=1))
            wtsp = ep(tc.tile_pool(name="wts", bufs=20))
            ktmpp = ep(tc.tile_pool(name="ktmp", bufs=3))
            ropep = ep(tc.tile_pool(name="ropet", bufs=3))
            kheadp = ep(tc.tile_pool(name="khead", bufs=2))
            ptp = ep(tc.tile_pool(name="pTp", bufs=3))
            vhp_p = ep(tc.tile_pool(name="vhp", bufs=4))
            smallp = ep(tc.tile_pool(name="small", bufs=4))
            outp = ep(tc.tile_pool(name="outsb", bufs=3))
            ps_mm = ep(tc.tile_pool(name="ps_mm", bufs=2, space="PSUM"))
            ps_s = ep(tc.tile_pool(name="ps_s", bufs=2, space="PSUM"))
            ps_av = ep(tc.tile_pool(name="ps_av", bufs=2, space="PSUM"))

            # ---- internal DRAM for collectives (split by head half) ----
            k_inh, k_gathh, v_inh, v_gathh = [], [], [], []
            for s in range(2):
                k_inh.append(dramp.tile([C // 2, NS], f32r, name=f"k_in{s}"))
                k_gathh.append(
                    dramp.tile([GB, C // 2, NS], f32r, name=f"k_gath{s}")
                )
                v_inh.append(
                    dramp.tile([NS, 8, HD + 1], f32r, name=f"v_in{s}")
                )
                v_gathh.append(
                    dramp.tile([GB, NS, 8, HD + 1], f32r, name=f"v_gath{s}")
                )

            # ---- constants / persistent loads ----
            cos_sb = constp.tile([128, NS], f32, name="cos_sb")
            nc.sync.dma_start(cos_sb[:, :], cos2[:, :])
            sin_sb = constp.tile([128, NS], f32, name="sin_sb")
            nc.sync.dma_start(sin_sb[:, :], sins2[:, :])
            bias_sb = constp.tile([128, C], f32, name="bias_sb")
            nc.sync.dma_start(bias_sb[:, :], biasb[:, :])
            onesf = constp.tile([128, 64], f32, name="onesf")
            nc.vector.memset(onesf[:, :], 1.0)

            xT_sb = xtp.tile([128, 8, NS], f32r, name="xT_sb")
            for cc in range(8):
                nc.sync.dma_start(
                    xT_sb[:, cc, :], xT[cc * 128:(cc + 1) * 128, :]
                )

            qT_sb = qtp.tile([128, 8, NS], f32r, name="qT_sb")
            aT_sb = atp.tile([128, 8, NS], f32r, name="aT_sb")

            def rope_chunk(psum, dst):
                """dst = psum*cos + rot32(psum)*signed_sin, all [128, NS]."""
                tmp = ropep.tile([128, NS], f32, name="tmp", tag="ropetmp")
                for lo in (0, 64):
                    nc.vector.tensor_mul(
                        tmp[lo:lo + 32, :],
                        psum[lo + 32:lo + 64, :],
                        sin_sb[lo:lo + 32, :],
                    )
                    nc.vector.tensor_mul(
                        tmp[lo + 32:lo + 64, :],
                        psum[lo:lo + 32, :],
                        sin_sb[lo + 32:lo + 64, :],
                    )
                nc.vector.tensor_mul(dst, psum, cos_sb[:, :])
                nc.vector.tensor_add(dst, dst, tmp[:, :])

            # ---- v (natural [i, dv]) and k^T, in head halves; AG each ----
            def ag(in_t, out_t, tag):
                if mock_ag:
                    for r in range(GB):
                        nc.gpsimd.dma_start(out_t[r, 0:32], in_t[0:32])
                else:
                    nc.gpsimd.collective_compute(
                        "AllGather",
                        mybir.AluOpType.bypass,
                        replica_groups=groups,
                        ins=[in_t.opt()],
                        outs=[out_t.opt()],
                    )

            for s in range(2):  # head half s: heads 8s..8s+7
                wv_tiles = []
                for cc in range(8):
                    w = wtsp.tile([128, 512], f32r, name="w", tag="wts")
                    nc.sync.dma_start(
                        w[:, :],
                        wvT[cc * 128:(cc + 1) * 128, s * 512:(s + 1) * 512],
                    )
                    wv_tiles.append(w)
                wk_tiles = []
                for cc in range(8):
                    w = wtsp.tile([128, 4, 128], f32r, name="w", tag="wts")
                    nc.scalar.dma_start(
                        w[:, :, :],
                        wqkT[
                            cc * 128:(cc + 1) * 128,
                            C + s * 512:C + (s + 1) * 512,
                        ].rearrange("p (m f) -> p m f", f=128),
                    )
                    wk_tiles.append(w)
                # v half
                for ic in range(4):
                    rows = slice(ic * 128, (ic + 1) * 128)
                    psum = ps_mm.tile([128, NS], f32, name="psum", tag="mm")
                    for cc in range(8):
                        mm(psum[:, :], xT_sb[:, cc, rows], wv_tiles[cc][:, :],
                           cc == 0, cc == 7)
                    vsb = outp.tile([128, 8, HD + 1], f32r, name="vsb", tag="osb")
                    nc.vector.tensor_copy(vsb[:, :, HD], onesf[:, 0:8])
                    nc.vector.tensor_copy(
                        vsb[:, :, 0:HD],
                        psum[:, :].rearrange("p (h d) -> p h d", d=HD),
                    )
                    nc.scalar.dma_start(v_inh[s][rows, :, :], vsb[:, :, :])
                ag(v_inh[s], v_gathh[s], f"v{s}")
                # k half
                for ml in range(4):
                    psum = ps_mm.tile([128, NS], f32, name="psum", tag="mm")
                    for cc in range(8):
                        mm(psum[:, :], wk_tiles[cc][:, ml, :], xT_sb[:, cc, :],
                           cc == 0, cc == 7)
                    kc = ktmpp.tile([128, NS], f32r, name="kc", tag="kc")
                    rope_chunk(psum[:, :], kc[:, :])
                    nc.scalar.dma_start(
                        k_inh[s][ml * 128:(ml + 1) * 128, :], kc[:, :]
                    )
                ag(k_inh[s], k_gathh[s], f"k{s}")

            # ---- q^T group (dq chunks 0..7) + rope, overlaps the gathers ----
            wq_tiles = {}
            for qh in range(2):
                for cc in range(8):
                    w = wtsp.tile([128, 4, 128], f32r, name="w", tag="wts")
                    nc.scalar.dma_start(
                        w[:, :, :],
                        wqkT[
                            cc * 128:(cc + 1) * 128, qh * 512:(qh + 1) * 512
                        ].rearrange("p (m f) -> p m f", f=128),
                    )
                    wq_tiles[(qh, cc)] = w
            for m in range(8):
                psum = ps_mm.tile([128, NS], f32, name="psum", tag="mm")
                for cc in range(8):
                    mm(psum[:, :], wq_tiles[(m // 4, cc)][:, m % 4, :],
                       xT_sb[:, cc, :], cc == 0, cc == 7)
                rope_chunk(psum[:, :], qT_sb[:, m, :])

            # ---- attention, head pairs (flash-style over key chunks) ----
            vg = {}
            for hp in range(H // 2):  # heads 2*hp, 2*hp+1
                if hp % 4 == 0:  # prefetch v for heads [8*g, 8*(g+1))
                    g = hp // 4
                    for r in range(GB):
                        vt = vhp_p.tile(
                            [128, GB, 8, HD + 1], f32r, name="vt", tag="vt"
                        )
                        for half in range(2):
                            eng = [nc.gpsimd, nc.sync][(r + half) % 2]
                            eng.dma_start(
                                vt[:, half * 2:(half + 1) * 2, :, :],
                                v_gathh[g][
                                    r, half * 256:(half + 1) * 256, :, :
                                ].rearrange("(a p) h d -> p a h d", p=128),
                            )
                        vg[r] = vt
                kh = kheadp.tile([128, GB, NS], f32r, name="kh", tag="khead")
                kh_engines = [nc.gpsimd, nc.sync, nc.gpsimd, nc.sync]
                for r in range(GB):
                    kh_engines[r].dma_start(
                        kh[:, r, :],
                        k_gathh[hp // 4][
                            r, (hp % 4) * 128:(hp % 4 + 1) * 128, :
                        ],
                    )
                for sub in range(2):  # head h = 2*hp + sub at partitions sub*64
                    h = 2 * hp + sub
                    lo = sub * 64
                    q_ap = qT_sb[lo:lo + 64, hp, :]
                    po = ps_av.tile([HD + 1, NS], f32, name="po", tag="av")
                    for jp in range(8):  # pairs of key chunks
                        jc0 = 2 * jp
                        ps2 = ps_s.tile([128, 2, NS], f32, name="ps2", tag="sc")
                        for u in range(2):
                            jc = jc0 + u
                            r, jl = jc // 4, jc % 4
                            mm(ps2[:, u, :],
                               kh[lo:lo + 64, r, jl * 128:(jl + 1) * 128],
                               q_ap, True, True)
                        pt = ptp.tile([128, 2, NS], f32r, name="pt", tag="pT")
                        nc.scalar.activation(
                            pt[:, :, :], ps2[:, :, :], AF.Exp, scale=SC
                        )
                        for u in range(2):
                            jc = jc0 + u
                            r, jl = jc // 4, jc % 4
                            mm(po[:, :],
                               vg[r][:, jl, 2 * (hp % 4) + sub, :],
                               pt[:, u, :], jc == 0, jc == 15)
                    # normalize: reciprocal of denom row, gpsimd broadcast
                    recip = smallp.tile([1, NS], f32, name="recip", tag="recip")
                    nc.vector.reciprocal(recip[:, :], po[HD:HD + 1, :])
                    rb = smallp.tile([64, NS], f32, name="rb", tag="rb")
                    nc.gpsimd.partition_broadcast(rb[:, :], recip[:, :])
                    nc.vector.tensor_mul(
                        aT_sb[lo:lo + 64, hp, :], po[0:HD, :], rb[:, :]
                    )

            # ---- projection, two passes: pass 0 (heads 0-7) can run while
            # the second half of attention is still in flight ----
            wp_tiles = {}
            for nn in range(2):
                for cc in range(8):
                    w = wtsp.tile([128, 512], f32r, name="w", tag="wts")
                    nc.sync.dma_start(
                        w[:, :],
                        wpT[cc * 128:(cc + 1) * 128, nn * 512:(nn + 1) * 512],
                    )
                    wp_tiles[(nn, cc)] = w
            pacc = qtp.tile([128, 8, 512], f32, name="pacc")
            for ic in range(4):
                rows = slice(ic * 128, (ic + 1) * 128)
                for nn in range(2):
                    psum = ps_mm.tile([128, NS], f32, name="psum", tag="mm")
                    for cc in range(4):
                        mm(psum[:, :], aT_sb[:, cc, rows],
                           wp_tiles[(nn, cc)][:, :], cc == 0, cc == 3)
                    nc.vector.tensor_add(
                        pacc[:, ic * 2 + nn, :],
                        psum[:, :],
                        bias_sb[:, nn * 512:(nn + 1) * 512],
                    )
            for ic in range(4):
                rows = slice(ic * 128, (ic + 1) * 128)
                for nn in range(2):
                    psum = ps_mm.tile([128, NS], f32, name="psum", tag="mm")
                    for cc in range(4, 8):
                        mm(psum[:, :], aT_sb[:, cc, rows],
                           wp_tiles[(nn, cc)][:, :], cc == 4, cc == 7)
                    osb = outp.tile([128, 512], f32, name="osb", tag="osb")
                    nc.vector.tensor_add(
                        osb[:, :], psum[:, :], pacc[:, ic * 2 + nn, :]
                    )
                    nc.sync.dma_start(out[rows, nn * 512:(nn + 1) * 512], osb[:, :])

    nc.compile()
    return nc


_NC_CACHE = {}


def _get_nc():
    if "nc" not in _NC_CACHE:
        _NC_CACHE["nc"] = build()
    return _NC_CACHE["nc"]


def make_in_maps(x, cos, sin, qkv_w, proj_w, proj_b):
    x = np.asarray(x, np.float32)
    cos = np.asarray(cos, np.float32)
    sin = np.asarray(sin, np.float32)
    qkv_w = np.asarray(qkv_w, np.float32)
    proj_w = np.asarray(proj_w, np.float32)
    proj_b = np.asarray(proj_b, np.float32)

    wqkT = np.ascontiguousarray(qkv_w[: 2 * C].T)        # [C, 2C]
    wvT = np.ascontiguousarray(qkv_w[2 * C:].T)          # [C, C]
    wpT = np.ascontiguousarray(proj_w.T)                 # [C, C]
    biasb = np.ascontiguousarray(np.broadcast_to(proj_b, (128, C)))
    sign = np.concatenate([-np.ones(32, np.float32), np.ones(32, np.float32)])

    in_maps = []
    for c in range(NCORES):
        b, r = c // GB, c % GB
        rows = slice(r * NS, (r + 1) * NS)
        xTc = np.ascontiguousarray(x[b, rows].T)          # [C, NS]
        cosT = cos[rows].T                                # [HD, NS]
        sinsT = (sin[rows] * sign).T                      # [HD, NS] signed
        cos2v = np.ascontiguousarray(np.concatenate([cosT, cosT], 0))   # [128, NS]
        sins2v = np.ascontiguousarray(np.concatenate([sinsT, sinsT], 0))
        in_maps.append(
            {
                "xT": xTc,
                "wqkT": wqkT,
                "wvT": wvT,
                "wpT": wpT,
                "cos2": cos2v,
                "sins2": sins2v,
                "biasb": biasb,
            }
        )
    return in_maps


def assemble(results):
    out = np.empty((B, N, C), np.float32)
    for c in range(NCORES):
        b, r = c // GB, c % GB
        out[b, r * NS:(r + 1) * NS] = results[c]["out"]
    return out


def kernel(x, cos, sin, qkv_w, proj_w, proj_b):
    from concourse.bass_utils import run_bass_kernel_spmd

    nc = _get_nc()
    in_maps = make_in_maps(x, cos, sin, qkv_w, proj_w, proj_b)
    res = run_bass_kernel_spmd(nc, in_maps, core_ids=list(range(NCORES)))
    return assemble(res.results)
